# revision 1
# baseline (speedup 1.0000x reference)
"""Cross-attention Trainium2 kernel (nn_CrossAttention_8486855377137).

Sharding (8 cores): core c = (batch b = c//2, head-group g = c%2).
Each core handles one batch and 4 of the 8 heads (Q/K/V projections
column-sharded by head, wo row-sharded). Full softmax over S on device;
host sums the two partial wo outputs per batch and adds wo bias.

Device dataflow (all transposed-world, zero on-device transposes):
  QT[gE=256, T]   = wqT.T @ xT     (+bq)
  KT[gE, S]       = wkT.T @ ctxT   (+bk)
  V[S, gE]        = ctxT.T @ wvT   (+bv)     (stored padded with ones col per head)
  scoresT[s,t]    = KT_h.T-slice matmuls, 2 heads row-packed per pair (K=64)
  expS            = exp(0.125 * scoresT)     (ScalarE, PSUM->SBUF bf16)
  O'[65, t]       = sum_s V'_h[s].T @ expS   (ones col -> row 64 = denominator)
  Ocat            = O'[0:64] * (1/den)       (normalized, bf16)
  yT[512, T]      = woT.T @ Ocat             (partial over head-group, fp32)
"""

import numpy as np
import ml_dtypes

import concourse.bass as bass
import concourse.bacc as bacc
import concourse.tile as tile
import concourse.mybir as mybir
from concourse.bass_utils import run_bass_kernel_spmd

BF16 = mybir.dt.bfloat16
F32 = mybir.dt.float32
EXP = mybir.ActivationFunctionType.Exp
ADD = mybir.AluOpType.add
MULT = mybir.AluOpType.mult
NPBF16 = ml_dtypes.bfloat16

# Problem constants (hardcoded per contract)
B, T, S = 4, 2048, 4096
E, KV = 512, 2048
H, D = 8, 64
GE = 256          # head-group embed width (4 heads x 64)
SCALE = D ** -0.5  # 0.125

N_CORES = 8


def _build_nc():
    nc = bacc.Bacc("TRN2", target_bir_lowering=False, debug=False)

    ctxT = nc.dram_tensor("ctxT", [KV, S], BF16, kind="ExternalInput")
    xT = nc.dram_tensor("xT", [E, T], BF16, kind="ExternalInput")
    wqT = nc.dram_tensor("wqT", [E, GE], BF16, kind="ExternalInput")
    wkT = nc.dram_tensor("wkT", [KV, GE], BF16, kind="ExternalInput")
    wvT = nc.dram_tensor("wvT", [KV, GE], BF16, kind="ExternalInput")
    woT = nc.dram_tensor("woT", [GE, E], BF16, kind="ExternalInput")
    bq = nc.dram_tensor("bq", [GE], F32, kind="ExternalInput")
    bk = nc.dram_tensor("bk", [GE], F32, kind="ExternalInput")
    bv = nc.dram_tensor("bv", [GE], F32, kind="ExternalInput")
    yT = nc.dram_tensor("yT", [E, T], F32, kind="ExternalOutput")

    with tile.TileContext(nc) as tc:
        _kernel_body(tc, nc, ctxT, xT, wqT, wkT, wvT, woT, bq, bk, bv, yT)
    nc.compile()
    return nc


def _kernel_body(tc, nc, ctxT, xT, wqT, wkT, wvT, woT, bq, bk, bv, yT):
    P = 128
    NT = T // 512        # 4 t-chunks
    NSC = S // P         # 32 s-chunks of 128
    NST = S // P         # 32 s-tiles for V
    KV_C = KV // P       # 16 contraction chunks for K/V proj
    E_C = E // P         # 4 contraction chunks for Q proj

    wts = tc.alloc_tile_pool(name="wts", bufs=1)
    persist = tc.alloc_tile_pool(name="persist", bufs=1)

    # ---- constant / weight loads ----
    wqT_sb = wts.tile([P, E_C, GE], BF16, tag="wqT")
    nc.sync.dma_start(wqT_sb, wqT.rearrange("(c p) m -> p c m", p=P))
    wkT_sb = wts.tile([P, KV_C, GE], BF16, tag="wkT")
    nc.sync.dma_start(wkT_sb, wkT.rearrange("(c p) m -> p c m", p=P))
    wvT_sb = wts.tile([P, KV_C, GE], BF16, tag="wvT")
    nc.sync.dma_start(wvT_sb, wvT.rearrange("(c p) m -> p c m", p=P))
    woT_sb = wts.tile([P, 2, E], BF16, tag="woT")
    nc.sync.dma_start(woT_sb, woT.rearrange("(c p) m -> p c m", p=P))
    xT_sb = wts.tile([P, E_C, T], BF16, tag="xT")
    nc.sync.dma_start(xT_sb, xT.rearrange("(c p) t -> p c t", p=P))

    bq_sb = wts.tile([P, 2], F32, tag="bq")
    nc.sync.dma_start(bq_sb, bq.rearrange("(c p) -> p c", p=P))
    bk_sb = wts.tile([P, 2], F32, tag="bk")
    nc.sync.dma_start(bk_sb, bk.rearrange("(c p) -> p c", p=P))
    # bv broadcast to all 128 partitions (fp32), used along free dim of V
    bv_bc = wts.tile([P, GE], F32, tag="bv_bc")
    bv_ap = bv.ap()
    bv_bcast_src = bass.AP(tensor=bv_ap.tensor, offset=bv_ap.offset,
                           ap=[[0, P]] + list(bv_ap.ap))
    nc.gpsimd.dma_start(out=bv_bc, in_=bv_bcast_src)

    # ---- persistent activation tiles ----
    QT_sb = [persist.tile([P, T], BF16, tag=f"QT{c}", name=f"QT{c}") for c in range(2)]
    KT_sb = [persist.tile([P, S], BF16, tag=f"KT{c}", name=f"KT{c}") for c in range(2)]
    # V padded: per s-tile [128, 4*65]; col h*65+64 holds ones
    VP = [persist.tile([P, 4 * 65], BF16, tag=f"VP{i}", name=f"VP{i}") for i in range(NST)]
    for i in range(NST):
        ones_view = VP[i].rearrange("p (h e) -> p h e", e=65)[:, :, 64:65]
        nc.vector.memset(ones_view, 1.0)
    OcatT = [persist.tile([P, T], BF16, tag=f"Ocat{c}", name=f"Ocat{c}") for c in range(2)]

    # ================= Phase 1: projections =================
    with tc.tile_pool(name="p1psum", bufs=2, space="PSUM") as p1ps, \
         tc.tile_pool(name="ctxpool", bufs=2) as ctxpool:

        # QT
        for m in range(2):
            for t in range(NT):
                ps = p1ps.tile([P, 512], F32, tag="qps")
                for c in range(E_C):
                    nc.tensor.matmul(
                        ps, wqT_sb[:, c, m * P:(m + 1) * P],
                        xT_sb[:, c, t * 512:(t + 1) * 512],
                        start=(c == 0), stop=(c == E_C - 1))
                nc.vector.tensor_scalar_add(
                    QT_sb[m][:, t * 512:(t + 1) * 512], ps, bq_sb[:, m:m + 1])

        # KT + V, streaming ctxT in 4 s-groups of 1024
        for sg in range(4):
            ctx_t = ctxpool.tile([P, KV_C, 1024], BF16, tag="ctx")
            nc.sync.dma_start(
                ctx_t,
                ctxT.rearrange("(c p) s -> p c s", p=P)[:, :, sg * 1024:(sg + 1) * 1024])
            for m in range(2):
                for n in range(2):
                    ps = p1ps.tile([P, 512], F32, tag="kps")
                    for c in range(KV_C):
                        nc.tensor.matmul(
                            ps, wkT_sb[:, c, m * P:(m + 1) * P],
                            ctx_t[:, c, n * 512:(n + 1) * 512],
                            start=(c == 0), stop=(c == KV_C - 1))
                    nc.vector.tensor_scalar_add(
                        KT_sb[m][:, sg * 1024 + n * 512: sg * 1024 + (n + 1) * 512],
                        ps, bk_sb[:, m:m + 1])
            for st in range(8):
                ps = p1ps.tile([P, GE], F32, tag="vps")
                for c in range(KV_C):
                    nc.tensor.matmul(
                        ps, ctx_t[:, c, st * P:(st + 1) * P], wvT_sb[:, c, :],
                        start=(c == 0), stop=(c == KV_C - 1))
                vp = VP[sg * 8 + st]
                nc.vector.tensor_tensor(
                    vp.rearrange("p (h e) -> p h e", e=65)[:, :, 0:64],
                    ps.rearrange("p (h e) -> p h e", e=64),
                    bv_bc.rearrange("p (h e) -> p h e", e=64),
                    ADD)

    # ================= Phase 2: attention + out-proj =================
    with tc.tile_pool(name="aps", bufs=1, space="PSUM") as aps, \
         tc.tile_pool(name="espool", bufs=6) as espool, \
         tc.tile_pool(name="npool", bufs=2) as npool, \
         tc.tile_pool(name="dramp", bufs=2, space="DRAM") as dramp, \
         tc.tile_pool(name="ystg", bufs=2) as ystg:

        def attn_unit(c2, t, inject=None):
            h0, h1 = 2 * c2, 2 * c2 + 1
            o_ps = [aps.tile([P, 512], F32, tag=f"o{j}", name=f"ops{j}", bufs=2)
                    for j in range(2)]
            for s in range(NSC):
                if inject and s % 4 == 1:
                    inject.pop(0)()
                slab = aps.tile([P, 1024], F32, tag="slab", bufs=2)
                nc.tensor.matmul(
                    slab[:, 0:512],
                    KT_sb[c2][0:64, s * P:(s + 1) * P],
                    QT_sb[c2][0:64, t * 512:(t + 1) * 512],
                    start=True, stop=True, skip_group_check=True)
                nc.tensor.matmul(
                    slab[:, 512:1024],
                    KT_sb[c2][64:128, s * P:(s + 1) * P],
                    QT_sb[c2][64:128, t * 512:(t + 1) * 512],
                    start=True, stop=True, skip_group_check=True)
                es = espool.tile([P, 1024], BF16, tag="es")
                nc.scalar.activation(es, slab, EXP, scale=SCALE)
                for j, h in enumerate((h0, h1)):
                    nc.tensor.matmul(
                        o_ps[j][:65],
                        VP[s][:, h * 65:(h + 1) * 65],
                        es[:, j * 512:(j + 1) * 512],
                        start=(s == 0), stop=(s == NSC - 1),
                        skip_group_check=True)
            # fast unnormalized eviction (frees PSUM quickly), then
            # normalization off the PE critical path
            for j in range(2):
                ps = o_ps[j]
                ou = npool.tile([65, 512], F32, tag="ou", bufs=4)
                nc.vector.tensor_copy(ou, ps[:65, :])
                # den -> DRAM -> broadcast to 64 partitions
                dscr = dramp.tile([1, 512], F32, tag="dscr")
                nc.sync.dma_start(dscr, ou[64:65, :])
                bc = npool.tile([64, 512], F32, tag="bc")
                bcast_src = bass.AP(tensor=dscr.tensor, offset=dscr.offset,
                                    ap=[[0, 64]] + list(dscr.ap[1:]))
                nc.gpsimd.dma_start(out=bc, in_=bcast_src)
                inv = npool.tile([64, 512], F32, tag="inv")
                nc.vector.reciprocal(inv, bc)
                if j == 0:
                    nc.vector.tensor_tensor(
                        OcatT[c2][0:64, t * 512:(t + 1) * 512],
                        ou[0:64, :], inv, MULT)
                else:
                    stg = npool.tile([64, 512], BF16, tag="stg")
                    nc.vector.tensor_tensor(stg, ou[0:64, :], inv, MULT)
                    nc.sync.dma_start(
                        OcatT[c2][64:128, t * 512:(t + 1) * 512], stg)

        yT_r = yT.rearrange("(m p) t -> p m t", p=P)

        def yproj_thunk(t, m):
            def run():
                ps = aps.tile([P, 1024], F32, tag="slab", bufs=2,
                              name=f"yps{m}")[:, 0:512]
                for c2 in range(2):
                    nc.tensor.matmul(
                        ps, woT_sb[:, c2, m * P:(m + 1) * P],
                        OcatT[c2][:, t * 512:(t + 1) * 512],
                        start=(c2 == 0), stop=(c2 == 1))
                yo = ystg.tile([P, 512], F32, tag="yo")
                nc.vector.tensor_copy(yo, ps)
                nc.sync.dma_start(yT_r[:, m, t * 512:(t + 1) * 512], yo)
            return run

        # yproj(t-1) emitted one unit late so eviction tails are done
        for t in range(NT):
            attn_unit(0, t)
            if t > 0:
                for m in range(E // P):
                    yproj_thunk(t - 1, m)()
            attn_unit(1, t)
        for m in range(E // P):
            yproj_thunk(NT - 1, m)()

    persist.release()
    wts.release()


_NC_CACHE = None
LAST_RESULT = None


def _get_nc():
    global _NC_CACHE
    if _NC_CACHE is None:
        _NC_CACHE = _build_nc()
    return _NC_CACHE


def kernel(x, context, wq_w, wq_b, wk_w, wk_b, wv_w, wv_b, wo_w, wo_b):
    x = np.asarray(x)
    context = np.asarray(context)
    nc = _get_nc()

    ctxT = [np.ascontiguousarray(context[b].T).astype(NPBF16) for b in range(B)]
    xT = [np.ascontiguousarray(x[b].T).astype(NPBF16) for b in range(B)]

    in_maps = []
    for c in range(N_CORES):
        b, g = c // 2, c % 2
        sl = slice(g * GE, (g + 1) * GE)
        in_maps.append({
            "ctxT": ctxT[b],
            "xT": xT[b],
            "wqT": np.ascontiguousarray(np.asarray(wq_w)[sl, :].T).astype(NPBF16),
            "wkT": np.ascontiguousarray(np.asarray(wk_w)[sl, :].T).astype(NPBF16),
            "wvT": np.ascontiguousarray(np.asarray(wv_w)[sl, :].T).astype(NPBF16),
            "woT": np.ascontiguousarray(np.asarray(wo_w)[:, sl].T).astype(NPBF16),
            "bq": np.ascontiguousarray(np.asarray(wq_b)[sl]).astype(np.float32),
            "bk": np.ascontiguousarray(np.asarray(wk_b)[sl]).astype(np.float32),
            "bv": np.ascontiguousarray(np.asarray(wv_b)[sl]).astype(np.float32),
        })

    res = run_bass_kernel_spmd(nc, in_maps, core_ids=list(range(N_CORES)))
    global LAST_RESULT
    LAST_RESULT = res
    outs = res.results

    wo_b = np.asarray(wo_b, dtype=np.float32)
    y = np.empty((B, T, E), dtype=np.float32)
    for b in range(B):
        yt = outs[2 * b]["yT"] + outs[2 * b + 1]["yT"]
        y[b] = yt.T + wo_b
    return y



# revision 4
# speedup vs baseline: 1.0187x; 1.0187x over previous
"""Cross-attention Trainium2 kernel (nn_CrossAttention_8486855377137).

Sharding (8 cores): core c = (batch b = c//2, head-group g = c%2).
Each core handles one batch and 4 of the 8 heads (Q/K/V projections
column-sharded by head, wo row-sharded). Full softmax over S on device;
host sums the two partial wo outputs per batch and adds wo bias.

v2: ACT-bound pipeline design.
  - K/V projections: fp8e4 DoubleRow matmuls (ctx + wk/wv in fp8,
    wk scaled x512 folded into the exp activation scale, wv scaled x32
    folded into the softmax normalization via ones-col = 32).
  - attn.V: fp8e4 DoubleRow (es written as fp8 by ScalarE with exp bias
    -1.5 to keep range < 240; V stored fp8 x32).
  - scores: bf16, two heads run concurrently on PE row-groups 0:64/64:128.
  - Projection phase fused into unit-0's s-loop so the ScalarE exp stream
    (the roofline: ~284us) starts ~15us into the kernel.
  - Softmax denominator via ones-column in V (row 64 of o_ps), reciprocal
    via reciprocal_approx_fast.
"""

import numpy as np
import ml_dtypes

import concourse.bass as bass
import concourse.bacc as bacc
import concourse.tile as tile
import concourse.mybir as mybir
from concourse.bass_utils import run_bass_kernel_spmd

BF16 = mybir.dt.bfloat16
F32 = mybir.dt.float32
FP8 = mybir.dt.float8e4
EXP = mybir.ActivationFunctionType.Exp
ADD = mybir.AluOpType.add
MULT = mybir.AluOpType.mult
DR = mybir.MatmulPerfMode.DoubleRow
NPBF16 = ml_dtypes.bfloat16
NPFP8 = ml_dtypes.float8_e4m3

# Problem constants (hardcoded per contract)
B, T, S = 4, 2048, 4096
E, KV = 512, 2048
H, D = 8, 64
GE = 256            # head-group embed width (4 heads x 64)
SCALE = D ** -0.5   # 0.125
WK_SCALE = 512.0    # wk prescale (fp8 denormal avoidance); folded into exp scale
WV_SCALE = 32.0     # wv prescale; folded via ones-col = 32 in V
EXP_BIAS = -1.5     # exp(x + bias): keeps es < 60 « fp8e4 max 240; cancels in softmax

N_CORES = 8
P = 128
NT = T // 512       # 4 t-chunks
NSC = S // P        # 32 s-tiles
NSG = 4             # s-groups of 1024
SG_TILES = NSC // NSG  # 8 s-tiles per group
KV_C = KV // P      # 16 contraction chunks for K/V proj
E_C = E // P        # 4 contraction chunks for Q proj


def _build_nc():
    nc = bacc.Bacc("TRN2", target_bir_lowering=False, debug=False)

    ctxT = nc.dram_tensor("ctxT", [KV, S], FP8, kind="ExternalInput")
    xT = nc.dram_tensor("xT", [E, T], BF16, kind="ExternalInput")
    wqT = nc.dram_tensor("wqT", [E, GE], BF16, kind="ExternalInput")
    wkT = nc.dram_tensor("wkT", [KV, GE], FP8, kind="ExternalInput")
    wvT = nc.dram_tensor("wvT", [KV, GE], FP8, kind="ExternalInput")
    woT = nc.dram_tensor("woT", [GE, E], BF16, kind="ExternalInput")
    bq = nc.dram_tensor("bq", [GE], F32, kind="ExternalInput")
    bk = nc.dram_tensor("bk", [GE], F32, kind="ExternalInput")   # x512
    bv = nc.dram_tensor("bv", [GE], F32, kind="ExternalInput")   # x32
    yT = nc.dram_tensor("yT", [E, T], F32, kind="ExternalOutput")

    with tile.TileContext(nc) as tc:
        _kernel_body(tc, nc, ctxT, xT, wqT, wkT, wvT, woT, bq, bk, bv, yT)
    nc.compile()
    return nc


def _kernel_body(tc, nc, ctxT, xT, wqT, wkT, wvT, woT, bq, bk, bv, yT):
    wts = tc.alloc_tile_pool(name="wts", bufs=1)
    persist = tc.alloc_tile_pool(name="persist", bufs=1)

    # ---- constant / weight loads ----
    wqT_sb = wts.tile([P, E_C, GE], BF16, tag="wqT")
    nc.sync.dma_start(wqT_sb, wqT.rearrange("(c p) m -> p c m", p=P))
    wkT_sb = wts.tile([P, KV_C, GE], FP8, tag="wkT")
    nc.sync.dma_start(wkT_sb, wkT.rearrange("(c p) m -> p c m", p=P))
    wvT_sb = wts.tile([P, KV_C, GE], FP8, tag="wvT")
    nc.sync.dma_start(wvT_sb, wvT.rearrange("(c p) m -> p c m", p=P))
    woT_sb = wts.tile([P, 2, E], BF16, tag="woT")
    nc.sync.dma_start(woT_sb, woT.rearrange("(c p) m -> p c m", p=P))
    xT_sb = wts.tile([P, E_C, T], BF16, tag="xT")
    nc.sync.dma_start(xT_sb, xT.rearrange("(c p) t -> p c t", p=P))

    bq_sb = wts.tile([P, 2], F32, tag="bq")
    nc.sync.dma_start(bq_sb, bq.rearrange("(c p) -> p c", p=P))
    bk_sb = wts.tile([P, 2], F32, tag="bk")
    nc.sync.dma_start(bk_sb, bk.rearrange("(c p) -> p c", p=P))
    # per-partition exp bias constant for the activation
    ebias_sb = wts.tile([P, 1], F32, tag="ebias")
    nc.vector.memset(ebias_sb, EXP_BIAS)
    # bv broadcast to all 128 partitions (fp32, x32), used along free dim of V
    bv_bc = wts.tile([P, GE], F32, tag="bv_bc")
    bv_ap = bv.ap()
    bv_bcast_src = bass.AP(tensor=bv_ap.tensor, offset=bv_ap.offset,
                           ap=[[0, P]] + list(bv_ap.ap))
    nc.gpsimd.dma_start(out=bv_bc, in_=bv_bcast_src)

    # ---- persistent activation tiles ----
    QT_sb = [persist.tile([P, T], BF16, tag=f"QT{c}", name=f"QT{c}") for c in range(2)]
    KT_sb = [persist.tile([P, S], BF16, tag=f"KT{c}", name=f"KT{c}") for c in range(2)]
    # V fp8, head-major: [128, head(4), s-tile(32), 80]; col 64 = 32.0 (den),
    # cols 65:80 pad so the s-tile stride (80 B) is 16B-aligned for DoubleRow.
    V_sb = persist.tile([P, 4, NSC, 80], FP8, tag="V", name="V")
    nc.vector.memset(V_sb[:, :, :, 64:65], WV_SCALE)
    OcatT = [persist.tile([P, T], BF16, tag=f"Ocat{c}", name=f"Ocat{c}") for c in range(2)]

    ctxT_r = ctxT.rearrange("(c p) s -> p c s", p=P)
    yT_r = yT.rearrange("(m p) t -> p m t", p=P)

    with tc.tile_pool(name="aps", bufs=1, space="PSUM") as aps, \
         tc.tile_pool(name="ctxpool", bufs=2) as ctxpool, \
         tc.tile_pool(name="espool", bufs=10) as espool, \
         tc.tile_pool(name="npool", bufs=2) as npool, \
         tc.tile_pool(name="dramp", bufs=4, space="DRAM") as dramp, \
         tc.tile_pool(name="ystg", bufs=2) as ystg:

        ctx_tiles = {}

        def ctx_dma(sg):
            ctx_t = ctxpool.tile([P, KV_C, 1024], FP8, tag="ctx")
            nc.sync.dma_start(ctx_t, ctxT_r[:, :, sg * 1024:(sg + 1) * 1024])
            ctx_tiles[sg] = ctx_t

        def k_group(sg):
            """KT[:, sg] for both c2 chunks; fp8 DoubleRow; bias-add x512."""
            ctx_t = ctx_tiles[sg]
            for m in range(2):
                ps = aps.tile([P, 1024], F32, tag="slab", bufs=2, name=f"kps{sg}{m}")
                for n in range(2):
                    for cp in range(KV_C // 2):
                        nc.tensor.matmul(
                            ps[:, n * 512:(n + 1) * 512],
                            wkT_sb[:, 2 * cp:2 * cp + 2, m * P:(m + 1) * P],
                            ctx_t[:, 2 * cp:2 * cp + 2, n * 512:(n + 1) * 512],
                            start=(cp == 0), stop=(cp == KV_C // 2 - 1),
                            perf_mode=DR, skip_group_check=True)
                nc.vector.tensor_scalar_add(
                    KT_sb[m][:, sg * 1024:(sg + 1) * 1024], ps, bk_sb[:, m:m + 1])

        def v_group(sg):
            """V[:, :, sg tiles] (x32 incl bias); fp8 DoubleRow."""
            ctx_t = ctx_tiles[sg]
            for half in range(2):
                ps = aps.tile([P, 1024], F32, tag="slab", bufs=2, name=f"vps{sg}{half}")
                for st4 in range(4):
                    st = half * 4 + st4
                    for cp in range(KV_C // 2):
                        nc.tensor.matmul(
                            ps[:, st4 * 256:(st4 + 1) * 256],
                            ctx_t[:, 2 * cp:2 * cp + 2, st * P:(st + 1) * P],
                            wvT_sb[:, 2 * cp:2 * cp + 2, :],
                            start=(cp == 0), stop=(cp == KV_C // 2 - 1),
                            perf_mode=DR, skip_group_check=True)
                for st4 in range(4):
                    st = half * 4 + st4
                    idx = sg * SG_TILES + st
                    nc.vector.tensor_tensor(
                        V_sb[:, :, idx, 0:64],
                        ps[:, st4 * 256:(st4 + 1) * 256].rearrange(
                            "p (h e) -> p h e", e=64),
                        bv_bc.rearrange("p (h e) -> p h e", e=64),
                        ADD)

        def q_proj():
            for c2 in range(2):
                for tp in range(2):
                    ps = aps.tile([P, 1024], F32, tag="slab", bufs=2,
                                  name=f"qps{c2}{tp}")
                    for tn in range(2):
                        t = tp * 2 + tn
                        for c in range(E_C):
                            nc.tensor.matmul(
                                ps[:, tn * 512:(tn + 1) * 512],
                                wqT_sb[:, c, c2 * P:(c2 + 1) * P],
                                xT_sb[:, c, t * 512:(t + 1) * 512],
                                start=(c == 0), stop=(c == E_C - 1),
                                skip_group_check=True)
                    nc.vector.tensor_scalar_add(
                        QT_sb[c2][:, tp * 1024:(tp + 1) * 1024], ps,
                        bq_sb[:, c2:c2 + 1])

        def attn_unit(c2, t, hooks=None, defer_attnv=False):
            """Scores + exp + attnV over full S; o_ps resident in PSUM."""
            hooks = hooks or {}
            tcols = slice(t * 512, (t + 1) * 512)
            o_ps = [aps.tile([P, 512], F32, tag=f"o{j}", name=f"o{c2}{t}{j}", bufs=2)
                    for j in range(2)]
            pending = []
            es_t = None

            def emit_attnv(p, es_tile):
                for j in range(2):
                    nc.tensor.matmul(
                        o_ps[j][:65],
                        V_sb[:, 2 * c2 + j, 2 * p:2 * p + 2, 0:65],
                        es_tile[:, j, :, :],
                        start=(p == 0), stop=(p == NSC // 2 - 1),
                        perf_mode=DR, skip_group_check=True)

            for s in range(NSC):
                for thunk in hooks.get(s, ()):
                    thunk()
                    for (pp, ee) in pending:
                        emit_attnv(pp, ee)
                    pending.clear()
                slab = aps.tile([P, 1024], F32, tag="slab", bufs=2,
                                name=f"sl{c2}{t}{s}")
                nc.tensor.matmul(
                    slab[:, 0:512],
                    KT_sb[c2][0:64, s * P:(s + 1) * P],
                    QT_sb[c2][0:64, tcols],
                    start=True, stop=True, skip_group_check=True)
                nc.tensor.matmul(
                    slab[:, 512:1024],
                    KT_sb[c2][64:128, s * P:(s + 1) * P],
                    QT_sb[c2][64:128, tcols],
                    start=True, stop=True, skip_group_check=True)
                if s % 2 == 0:
                    es_t = espool.tile([P, 2, 2, 512], FP8, tag="es", bufs=10)
                nc.scalar.activation(
                    es_t[:, :, s % 2, :],
                    slab.rearrange("p (h n) -> p h n", h=2),
                    EXP, scale=SCALE / WK_SCALE, bias=ebias_sb[:, 0:1])
                if s % 2 == 1:
                    p = s // 2
                    if defer_attnv:
                        pending.append((p, es_t))
                    else:
                        emit_attnv(p, es_t)
            for thunk in hooks.get(NSC, ()):
                thunk()
                for (pp, ee) in pending:
                    emit_attnv(pp, ee)
                pending.clear()

            # ---- eviction + normalization ----
            inv = []
            for j in range(2):
                ou = npool.tile([65, 512], F32, tag=f"ou{j}", bufs=2)
                nc.vector.tensor_copy(ou, o_ps[j][:65, :])
                dscr = dramp.tile([1, 512], F32, tag="dscr")
                nc.sync.dma_start(dscr, ou[64:65, :])
                bc = npool.tile([64, 512], F32, tag=f"bc{j}", bufs=2)
                bcast_src = bass.AP(tensor=dscr.tensor, offset=dscr.offset,
                                    ap=[[0, 64]] + list(dscr.ap[1:]))
                nc.gpsimd.dma_start(out=bc, in_=bcast_src)
                iv = npool.tile([64, 512], F32, tag=f"inv{j}", bufs=2)
                nc.vector.reciprocal_approx_fast(out=iv, in_=bc)
                inv.append(iv)
                if j == 0:
                    nc.vector.tensor_tensor(
                        OcatT[c2][0:64, tcols], ou[0:64, :], iv, MULT)
                else:
                    stg = npool.tile([64, 512], BF16, tag="stg", bufs=2)
                    nc.vector.tensor_tensor(stg, ou[0:64, :], iv, MULT)
                    nc.sync.dma_start(OcatT[c2][64:128, tcols], stg)

        def y_proj(t):
            tcols = slice(t * 512, (t + 1) * 512)
            for mp in range(2):
                ps = aps.tile([P, 1024], F32, tag="slab", bufs=2, name=f"yps{t}{mp}")
                for mn in range(2):
                    m = mp * 2 + mn
                    for c2 in range(2):
                        nc.tensor.matmul(
                            ps[:, mn * 512:(mn + 1) * 512],
                            woT_sb[:, c2, m * P:(m + 1) * P],
                            OcatT[c2][:, tcols],
                            start=(c2 == 0), stop=(c2 == 1),
                            skip_group_check=True)
                yo = ystg.tile([P, 1024], F32, tag="yo")
                nc.vector.tensor_copy(yo, ps)
                for mn in range(2):
                    m = mp * 2 + mn
                    nc.sync.dma_start(yT_r[:, m, tcols],
                                      yo[:, mn * 512:(mn + 1) * 512])

        # ================= emission schedule =================
        ctx_dma(0)
        k_group(0)
        q_proj()

        # unit (0,0) chases K/V production: at each s-group boundary emit
        # V(g-1) + ctx/K(g); its attnV lags one group behind the exp stream.
        hooks = {}
        for g in range(1, NSG):
            hooks[g * SG_TILES] = [lambda g=g: (ctx_dma(g), v_group(g - 1),
                                                k_group(g))]
        hooks[NSC] = [lambda: v_group(NSG - 1)]
        attn_unit(0, 0, hooks=hooks, defer_attnv=True)
        attn_unit(1, 0)
        for t in range(1, NT):
            attn_unit(0, t)
            y_proj(t - 1)
            attn_unit(1, t)
        y_proj(NT - 1)

    persist.release()
    wts.release()


_NC_CACHE = None
LAST_RESULT = None


def _get_nc():
    global _NC_CACHE
    if _NC_CACHE is None:
        _NC_CACHE = _build_nc()
    return _NC_CACHE


def kernel(x, context, wq_w, wq_b, wk_w, wk_b, wv_w, wv_b, wo_w, wo_b):
    x = np.asarray(x)
    context = np.asarray(context)
    nc = _get_nc()

    ctxT = [np.ascontiguousarray(context[b].T).astype(NPFP8) for b in range(B)]
    xT = [np.ascontiguousarray(x[b].T).astype(NPBF16) for b in range(B)]

    in_maps = []
    for c in range(N_CORES):
        b, g = c // 2, c % 2
        sl = slice(g * GE, (g + 1) * GE)
        in_maps.append({
            "ctxT": ctxT[b],
            "xT": xT[b],
            "wqT": np.ascontiguousarray(np.asarray(wq_w)[sl, :].T).astype(NPBF16),
            "wkT": np.ascontiguousarray(
                np.asarray(wk_w)[sl, :].T * WK_SCALE).astype(NPFP8),
            "wvT": np.ascontiguousarray(
                np.asarray(wv_w)[sl, :].T * WV_SCALE).astype(NPFP8),
            "woT": np.ascontiguousarray(np.asarray(wo_w)[:, sl].T).astype(NPBF16),
            "bq": np.ascontiguousarray(np.asarray(wq_b)[sl]).astype(np.float32),
            "bk": np.ascontiguousarray(
                np.asarray(wk_b)[sl] * WK_SCALE).astype(np.float32),
            "bv": np.ascontiguousarray(
                np.asarray(wv_b)[sl] * WV_SCALE).astype(np.float32),
        })

    res = run_bass_kernel_spmd(nc, in_maps, core_ids=list(range(N_CORES)))
    global LAST_RESULT
    LAST_RESULT = res
    outs = res.results

    wo_b = np.asarray(wo_b, dtype=np.float32)
    y = np.empty((B, T, E), dtype=np.float32)
    for b in range(B):
        yt = outs[2 * b]["yT"] + outs[2 * b + 1]["yT"]
        y[b] = yt.T + wo_b
    return y


# revision 5
# speedup vs baseline: 1.1292x; 1.1085x over previous
"""Cross-attention Trainium2 kernel (nn_CrossAttention_8486855377137).

Sharding (8 cores): core c = (batch b = c//2, head-group g = c%2).
Each core handles one batch and 4 of the 8 heads (Q/K/V projections
column-sharded by head, wo row-sharded). Full softmax over S on device;
host sums the two partial wo outputs per batch and adds wo bias.

v3: ACT-bound fused pipeline.
  - ScalarE exp stream (33.5M elem/core ~= 284us) is the roofline; the
    whole schedule exists to keep it dense from ~15us onward.
  - K path in fp8e4 DoubleRow (ctx fp8 + wk fp8 x512, scale folded into
    the exp activation scale). V path + es stay bf16 (fp8 there costs
    ~1e-2 rel err each, too close to the 2e-2 gate).
  - scores: bf16, two heads concurrent on PE row-groups 0:64/64:128.
  - K/V production is interleaved with the first TWO attention units
    (generator-based chase, half-s-group granularity) so ACT never
    starves; those units defer attnV one half-group behind exp.
  - Softmax denominator via ones-column in V; reciprocal_approx_fast.
"""

import numpy as np
import ml_dtypes

import concourse.bass as bass
import concourse.bacc as bacc
import concourse.tile as tile
import concourse.mybir as mybir
from concourse.bass_utils import run_bass_kernel_spmd

BF16 = mybir.dt.bfloat16
F32 = mybir.dt.float32
FP8 = mybir.dt.float8e4
EXP = mybir.ActivationFunctionType.Exp
ADD = mybir.AluOpType.add
MULT = mybir.AluOpType.mult
DR = mybir.MatmulPerfMode.DoubleRow
NPBF16 = ml_dtypes.bfloat16
NPFP8 = ml_dtypes.float8_e4m3

# Problem constants (hardcoded per contract)
B, T, S = 4, 2048, 4096
E, KV = 512, 2048
H, D = 8, 64
GE = 256            # head-group embed width (4 heads x 64)
SCALE = D ** -0.5   # 0.125
WK_SCALE = 512.0    # wk prescale (fp8 denormal avoidance); folded into exp scale
EXP_BIAS = -1.5     # exp(x + bias): cancels in softmax, shrinks es range

N_CORES = 8
P = 128
NT = T // 512       # 4 t-chunks
NSC = S // P        # 32 s-tiles
SGT = 4             # s-tiles per chase production step (512 cols)
NSG = NSC // SGT    # 8 production steps
KV_C = KV // P      # 16 contraction chunks for K/V proj
E_C = E // P        # 4 contraction chunks for Q proj


def _build_nc():
    nc = bacc.Bacc("TRN2", target_bir_lowering=False, debug=False)

    ctx8 = nc.dram_tensor("ctx8", [KV, S], FP8, kind="ExternalInput")
    ctxb = nc.dram_tensor("ctxb", [KV, S], BF16, kind="ExternalInput")
    xT = nc.dram_tensor("xT", [E, T], BF16, kind="ExternalInput")
    wqT = nc.dram_tensor("wqT", [E, GE], BF16, kind="ExternalInput")
    wkT = nc.dram_tensor("wkT", [KV, GE], FP8, kind="ExternalInput")
    wvT = nc.dram_tensor("wvT", [KV, GE], BF16, kind="ExternalInput")
    woT = nc.dram_tensor("woT", [GE, E], BF16, kind="ExternalInput")
    bq = nc.dram_tensor("bq", [GE], F32, kind="ExternalInput")
    bk = nc.dram_tensor("bk", [GE], F32, kind="ExternalInput")   # x512
    bv = nc.dram_tensor("bv", [GE], F32, kind="ExternalInput")
    yT = nc.dram_tensor("yT", [E, T], F32, kind="ExternalOutput")

    with tile.TileContext(nc) as tc:
        _kernel_body(tc, nc, ctx8, ctxb, xT, wqT, wkT, wvT, woT, bq, bk, bv, yT)
    nc.compile()
    return nc


def _kernel_body(tc, nc, ctx8, ctxb, xT, wqT, wkT, wvT, woT, bq, bk, bv, yT):
    wts = tc.alloc_tile_pool(name="wts", bufs=1)
    persist = tc.alloc_tile_pool(name="persist", bufs=1)

    # ---- constant / weight loads ----
    wqT_sb = wts.tile([P, E_C, GE], BF16, tag="wqT")
    nc.sync.dma_start(wqT_sb, wqT.rearrange("(c p) m -> p c m", p=P))
    wkT_sb = wts.tile([P, KV_C, GE], FP8, tag="wkT")
    nc.sync.dma_start(wkT_sb, wkT.rearrange("(c p) m -> p c m", p=P))
    wvT_sb = wts.tile([P, KV_C, GE], BF16, tag="wvT")
    nc.sync.dma_start(wvT_sb, wvT.rearrange("(c p) m -> p c m", p=P))
    woT_sb = wts.tile([P, 2, E], BF16, tag="woT")
    nc.sync.dma_start(woT_sb, woT.rearrange("(c p) m -> p c m", p=P))
    xT_sb = wts.tile([P, E_C, T], BF16, tag="xT")
    nc.sync.dma_start(xT_sb, xT.rearrange("(c p) t -> p c t", p=P))

    bq_sb = wts.tile([P, 2], F32, tag="bq")
    nc.sync.dma_start(bq_sb, bq.rearrange("(c p) -> p c", p=P))
    bk_sb = wts.tile([P, 2], F32, tag="bk")
    nc.sync.dma_start(bk_sb, bk.rearrange("(c p) -> p c", p=P))
    ebias_sb = wts.tile([P, 1], F32, tag="ebias")
    nc.vector.memset(ebias_sb, EXP_BIAS)
    # bv broadcast to all 128 partitions, used along free dim of V
    bv_bc = wts.tile([P, GE], F32, tag="bv_bc")
    bv_ap = bv.ap()
    bv_bcast_src = bass.AP(tensor=bv_ap.tensor, offset=bv_ap.offset,
                           ap=[[0, P]] + list(bv_ap.ap))
    nc.gpsimd.dma_start(out=bv_bc, in_=bv_bcast_src)

    # ---- persistent activation tiles ----
    QT_sb = [persist.tile([P, T], BF16, tag=f"QT{c}", name=f"QT{c}") for c in range(2)]
    KT_sb = [persist.tile([P, S], BF16, tag=f"KT{c}", name=f"KT{c}") for c in range(2)]
    # V bf16, head-major: [128, head(4), s-tile(32), 65]; col 64 = 1.0 (den)
    V_sb = persist.tile([P, 4, NSC, 65], BF16, tag="V", name="V")
    nc.vector.memset(V_sb[:, :, :, 64:65], 1.0)
    OcatT = [persist.tile([P, T], BF16, tag=f"Ocat{c}", name=f"Ocat{c}") for c in range(2)]

    ctx8_r = ctx8.rearrange("(c p) s -> p c s", p=P)
    ctxb_r = ctxb.rearrange("(c p) s -> p c s", p=P)
    yT_r = yT.rearrange("(m p) t -> p m t", p=P)

    with tc.tile_pool(name="aps", bufs=1, space="PSUM") as aps, \
         tc.tile_pool(name="c8pool", bufs=2) as c8pool, \
         tc.tile_pool(name="cbpool", bufs=2) as cbpool, \
         tc.tile_pool(name="espool", bufs=12) as espool, \
         tc.tile_pool(name="npool", bufs=2) as npool, \
         tc.tile_pool(name="dramp", bufs=4, space="DRAM") as dramp, \
         tc.tile_pool(name="ystg", bufs=2) as ystg:

        ctx8_tiles = {}
        ctxb_tiles = {}

        def ctx_dma(sg):
            """Fetch ctx half-group sg (512 cols) in fp8 (K) and bf16 (V)."""
            cols = slice(sg * 512, (sg + 1) * 512)
            t8 = c8pool.tile([P, KV_C, 512], FP8, tag="c8")
            nc.sync.dma_start(t8, ctx8_r[:, :, cols])
            ctx8_tiles[sg] = t8
            tb = cbpool.tile([P, KV_C, 512], BF16, tag="cb")
            nc.sync.dma_start(tb, ctxb_r[:, :, cols])
            ctxb_tiles[sg] = tb

        def k_group(sg):
            """KT[:, sg cols] for both c2 chunks; fp8 DoubleRow; x512."""
            ctx_t = ctx8_tiles.pop(sg)
            ps = aps.tile([P, 1024], F32, tag="slab", bufs=2, name=f"kps{sg}")
            for m in range(2):
                for cp in range(KV_C // 2):
                    nc.tensor.matmul(
                        ps[:, m * 512:(m + 1) * 512],
                        wkT_sb[:, 2 * cp:2 * cp + 2, m * P:(m + 1) * P],
                        ctx_t[:, 2 * cp:2 * cp + 2, :],
                        start=(cp == 0), stop=(cp == KV_C // 2 - 1),
                        perf_mode=DR, skip_group_check=True)
            for m in range(2):
                nc.vector.tensor_scalar_add(
                    KT_sb[m][:, sg * 512:(sg + 1) * 512],
                    ps[:, m * 512:(m + 1) * 512], bk_sb[:, m:m + 1])

        def v_group(sg):
            """V s-tiles of half-group sg; bf16."""
            ctx_t = ctxb_tiles.pop(sg)
            ps = aps.tile([P, 1024], F32, tag="slab", bufs=2, name=f"vps{sg}")
            for st4 in range(SGT):
                for c in range(KV_C):
                    nc.tensor.matmul(
                        ps[:, st4 * 256:(st4 + 1) * 256],
                        ctx_t[:, c, st4 * P:(st4 + 1) * P],
                        wvT_sb[:, c, :],
                        start=(c == 0), stop=(c == KV_C - 1),
                        skip_group_check=True)
            for st4 in range(SGT):
                idx = sg * SGT + st4
                nc.vector.tensor_tensor(
                    V_sb[:, :, idx, 0:64],
                    ps[:, st4 * 256:(st4 + 1) * 256].rearrange(
                        "p (h e) -> p h e", e=64),
                    bv_bc.rearrange("p (h e) -> p h e", e=64),
                    ADD)

        def q_proj():
            for c2 in range(2):
                for tp in range(2):
                    ps = aps.tile([P, 1024], F32, tag="slab", bufs=2,
                                  name=f"qps{c2}{tp}")
                    for tn in range(2):
                        t = tp * 2 + tn
                        for c in range(E_C):
                            nc.tensor.matmul(
                                ps[:, tn * 512:(tn + 1) * 512],
                                wqT_sb[:, c, c2 * P:(c2 + 1) * P],
                                xT_sb[:, c, t * 512:(t + 1) * 512],
                                start=(c == 0), stop=(c == E_C - 1),
                                skip_group_check=True)
                    nc.vector.tensor_scalar_add(
                        QT_sb[c2][:, tp * 1024:(tp + 1) * 1024], ps,
                        bq_sb[:, c2:c2 + 1])

        def attn_unit(c2, t, chase=False):
            """Generator: scores + exp + attnV over full S, then normalize.

            When chase=True, yields at every SGT s-tile boundary (before
            the segment that needs fresh K) so the driver can emit K/V
            production; attnV lags one segment (V not yet produced)."""
            tcols = slice(t * 512, (t + 1) * 512)
            o_ps = [aps.tile([P, 512], F32, tag=f"o{j}", name=f"o{c2}{t}{j}",
                             bufs=2) for j in range(2)]
            pending = []

            def emit_attnv(s, es_tile):
                for j in range(2):
                    nc.tensor.matmul(
                        o_ps[j][:65],
                        V_sb[:, 2 * c2 + j, s, 0:65],
                        es_tile[:, j * 512:(j + 1) * 512],
                        start=(s == 0), stop=(s == NSC - 1),
                        skip_group_check=True)

            for s in range(NSC):
                if chase and s % SGT == 0:
                    yield s
                    for (ss, ee) in pending:
                        emit_attnv(ss, ee)
                    pending.clear()
                slab = aps.tile([P, 1024], F32, tag="slab", bufs=2,
                                name=f"sl{c2}{t}{s}")
                nc.tensor.matmul(
                    slab[:, 0:512],
                    KT_sb[c2][0:64, s * P:(s + 1) * P],
                    QT_sb[c2][0:64, tcols],
                    start=True, stop=True, skip_group_check=True)
                nc.tensor.matmul(
                    slab[:, 512:1024],
                    KT_sb[c2][64:128, s * P:(s + 1) * P],
                    QT_sb[c2][64:128, tcols],
                    start=True, stop=True, skip_group_check=True)
                es_t = espool.tile([P, 1024], BF16, tag="es", bufs=12)
                nc.scalar.activation(es_t, slab, EXP, scale=SCALE / WK_SCALE,
                                     bias=ebias_sb[:, 0:1])
                if chase:
                    pending.append((s, es_t))
                else:
                    emit_attnv(s, es_t)
            if chase:
                yield NSC
                for (ss, ee) in pending:
                    emit_attnv(ss, ee)
                pending.clear()

            # ---- eviction + normalization ----
            for j in range(2):
                ou = npool.tile([65, 512], F32, tag=f"ou{j}", bufs=2)
                nc.vector.tensor_copy(ou, o_ps[j][:65, :])
                dscr = dramp.tile([1, 512], F32, tag="dscr")
                nc.sync.dma_start(dscr, ou[64:65, :])
                bc = npool.tile([64, 512], F32, tag=f"bc{j}", bufs=2)
                bcast_src = bass.AP(tensor=dscr.tensor, offset=dscr.offset,
                                    ap=[[0, 64]] + list(dscr.ap[1:]))
                nc.gpsimd.dma_start(out=bc, in_=bcast_src)
                iv = npool.tile([64, 512], F32, tag=f"inv{j}", bufs=2)
                nc.vector.reciprocal_approx_fast(out=iv, in_=bc)
                if j == 0:
                    nc.vector.tensor_tensor(
                        OcatT[c2][0:64, tcols], ou[0:64, :], iv, MULT)
                else:
                    stg = npool.tile([64, 512], BF16, tag="stg", bufs=2)
                    nc.vector.tensor_tensor(stg, ou[0:64, :], iv, MULT)
                    nc.sync.dma_start(OcatT[c2][64:128, tcols], stg)

        def y_proj(t):
            tcols = slice(t * 512, (t + 1) * 512)
            for mp in range(2):
                ps = aps.tile([P, 1024], F32, tag="slab", bufs=2, name=f"yps{t}{mp}")
                for mn in range(2):
                    m = mp * 2 + mn
                    for c2 in range(2):
                        nc.tensor.matmul(
                            ps[:, mn * 512:(mn + 1) * 512],
                            woT_sb[:, c2, m * P:(m + 1) * P],
                            OcatT[c2][:, tcols],
                            start=(c2 == 0), stop=(c2 == 1),
                            skip_group_check=True)
                yo = ystg.tile([P, 1024], F32, tag="yo")
                nc.vector.tensor_copy(yo, ps)
                for mn in range(2):
                    m = mp * 2 + mn
                    nc.sync.dma_start(yT_r[:, m, tcols],
                                      yo[:, mn * 512:(mn + 1) * 512])

        # ================= emission schedule =================
        # Chase phase: units (0,0) and (1,0) interleave with K/V production.
        ctx_dma(0)
        k_group(0)
        q_proj()

        u0 = attn_unit(0, 0, chase=True)
        u1 = attn_unit(1, 0, chase=True)
        next(u0)   # emits nothing yet (yield at s=0)
        next(u1)
        for g in range(NSG):
            # produce for the segment the units are about to consume:
            # K(g) already done for g=0; V(g) now; prefetch + K for g+1.
            if g + 1 < NSG:
                ctx_dma(g + 1)
            v_group(g)
            if g + 1 < NSG:
                k_group(g + 1)
            next(u0)   # scores/exp for segment g (+ attnV of segment g-1)
            next(u1)
        for u in (u0, u1):
            try:
                while True:
                    next(u)
            except StopIteration:
                pass

        for t in range(1, NT):
            for _ in attn_unit(0, t):
                pass
            y_proj(t - 1)
            for _ in attn_unit(1, t):
                pass
        y_proj(NT - 1)

    persist.release()
    wts.release()


_NC_CACHE = None
LAST_RESULT = None


def _get_nc():
    global _NC_CACHE
    if _NC_CACHE is None:
        _NC_CACHE = _build_nc()
    return _NC_CACHE


def kernel(x, context, wq_w, wq_b, wk_w, wk_b, wv_w, wv_b, wo_w, wo_b):
    x = np.asarray(x)
    context = np.asarray(context)
    nc = _get_nc()

    ctxT = [np.ascontiguousarray(context[b].T) for b in range(B)]
    ctx8 = [c.astype(NPFP8) for c in ctxT]
    ctxb = [c.astype(NPBF16) for c in ctxT]
    xT = [np.ascontiguousarray(x[b].T).astype(NPBF16) for b in range(B)]

    in_maps = []
    for c in range(N_CORES):
        b, g = c // 2, c % 2
        sl = slice(g * GE, (g + 1) * GE)
        in_maps.append({
            "ctx8": ctx8[b],
            "ctxb": ctxb[b],
            "xT": xT[b],
            "wqT": np.ascontiguousarray(np.asarray(wq_w)[sl, :].T).astype(NPBF16),
            "wkT": np.ascontiguousarray(
                np.asarray(wk_w)[sl, :].T * WK_SCALE).astype(NPFP8),
            "wvT": np.ascontiguousarray(np.asarray(wv_w)[sl, :].T).astype(NPBF16),
            "woT": np.ascontiguousarray(np.asarray(wo_w)[:, sl].T).astype(NPBF16),
            "bq": np.ascontiguousarray(np.asarray(wq_b)[sl]).astype(np.float32),
            "bk": np.ascontiguousarray(
                np.asarray(wk_b)[sl] * WK_SCALE).astype(np.float32),
            "bv": np.ascontiguousarray(np.asarray(wv_b)[sl]).astype(np.float32),
        })

    res = run_bass_kernel_spmd(nc, in_maps, core_ids=list(range(N_CORES)))
    global LAST_RESULT
    LAST_RESULT = res
    outs = res.results

    wo_b = np.asarray(wo_b, dtype=np.float32)
    y = np.empty((B, T, E), dtype=np.float32)
    for b in range(B):
        yt = outs[2 * b]["yT"] + outs[2 * b + 1]["yT"]
        y[b] = yt.T + wo_b
    return y
